# revision 18
# baseline (speedup 1.0000x reference)
"""NF4-quantized LoRA linear layer on 8 Trainium2 NeuronCores.

Computation (reference):
    w = NF4_TABLE[w_codes] * w_scales[block-expanded]        # [O, I]
    out = x @ w.T + (alpha/rank) * (x @ lora_a.T) @ lora_b.T # [B, S, O]

Strategy (v7):
  - Tensor-parallel split of the output dim across 8 cores (O_SH = 512 each).
    Every core sees all of x; no collectives; host concatenates outputs.
  - LoRA folded into the weights: W_eff = dequant + (alpha/rank) * la.T @ lb.
    The steady-state loop is a single dense f16 matmul stream at the PE
    roofline (216 ns per 128x128x512 matmul).
  - NF4 dequant is a polynomial in u = (2c-15)/16 (Horner), each step split
    into tensor_scalar (4x DVE mode) + tensor_tensor (2x).  Degree 9 for
    the bulk; degree 5 for i-tiles 0..3 so the PE starts earlier (matching
    BETA folded per-tile into the shipped scales).  Dequant runs per i-tile
    ([128,512] chunks, ~5us DVE each), emission spread through the phase
    loops so the strict-FIFO DVE queue never blocks the partial-add stream
    longer than the PSUM-pool slack.
  - Partial accumulators live in SBUF (f16), no DRAM round-trip. 4 m-loop
    phases over i (4/8/8/12 tiles).
  - Queue discipline (everything is strict-FIFO per engine): x blocks ride
    the Sync hwdge queue alone; u/st and output DMAs ride the Scalar hwdge
    queue; per-phase x pools with 4/2/2/4-deep lookahead plus pre-issued
    first blocks of the next phase keep the x latency chain off the PE.
  - Output is written f16 (host upcasts) halving write traffic; 4 ev
    buffers decouple the final DVE adds from out-DMA completion.
"""

import numpy as np

import concourse.mybir as mybir
import concourse.tile as tile
from concourse import bacc
from concourse.bass_utils import run_bass_kernel_spmd

B, S, I, O, R, BLK = 4, 2048, 4096, 4096, 16, 64
M = B * S                      # 8192 token rows
N_CORES = 8
O_SH = O // N_CORES            # 512 output cols per core
IT = I // 128                  # 32 contraction tiles
MT = M // 128                  # 64 row tiles
PHASES = [(0, 4), (4, 12), (12, 20), (20, 32)]  # i-tile ranges per phase
# i-tile -> (phase index, mt position) where its dequant chunk is emitted
TILE_EMIT = {}
for _j, _t in enumerate(range(4, 12)):
    TILE_EMIT[_t] = (0, 2 + 5 * _j)
for _j, _t in enumerate(range(12, 22)):
    TILE_EMIT[_t] = (1, 2 + 5 * _j)
for _j, _t in enumerate(range(22, 32)):
    TILE_EMIT[_t] = (2, 2 + 5 * _j)
FOLD_UPFRONT = 10              # LoRA-fold matmuls emitted before the m-loop
XBLKS = [4, 4, 4, 2]           # row tiles per x block DMA, per phase
XBUFS = [3, 2, 2, 4]           # x pool depth per phase
LORA_SCALE = 2.0               # alpha / rank

# Polynomial fits of NF4_TABLE at u = (2c-15)/16, Horner form with per-step
# split rounding: acc2 = f16(acc + a_j); acc = f16(acc2 * u);
# t = BETA*(acc + GOB).  BETA is folded into the shipped scales (per tile).
# Degree 7 (max table err 2.2e-3): 4 fewer DVE ops per tile than deg 9 for
# nearly identical output error at this error budget.
A_COEFS = [-0.15942862017839354, -0.7764774849226515, 0.21027965499734694,
           0.4876514257419115, -0.11881474113863193, 0.8185641529184308]
GOB = 0.049453697193569526
BETA = 0.8032323446473313
# degree-5 variant for i-tiles 0..3 (max table err 8.9e-3 there)
A5_COEFS = [0.027231205341471554, -0.30620367701201784,
            -0.07023236639251894, 1.0711502059105766]
GOB5 = 0.05403876741036842
BETA5 = 0.6774649525521037
N_DEG5_TILES = 4

F16 = mybir.dt.float16
F32 = mybir.dt.float32
ALU = mybir.AluOpType

F16_NP = np.float16


def _build_nc():
    nc = bacc.Bacc("TRN2", target_bir_lowering=False, debug=False,
                   num_devices=N_CORES)

    xt = nc.dram_tensor("xt", [128, MT, IT, 128], F16, kind="ExternalInput")
    uc = nc.dram_tensor("uc", [128, IT, O_SH], F16, kind="ExternalInput")
    sc = nc.dram_tensor("sc", [128, IT, O_SH], F16, kind="ExternalInput")
    la = nc.dram_tensor("la", [R, I], F16, kind="ExternalInput")
    lb = nc.dram_tensor("lb", [R, O_SH], F16, kind="ExternalInput")
    out = nc.dram_tensor("out", [M, O_SH], F16, kind="ExternalOutput")

    with tile.TileContext(nc) as tc:
        with (
            tc.tile_pool(name="cpool", bufs=1) as cpool,
            tc.tile_pool(name="wpool", bufs=IT) as wpool,
            tc.tile_pool(name="wlpool", bufs=FOLD_UPFRONT) as wlpool,
            tc.tile_pool(name="dqio", bufs=3) as dqio,
            tc.tile_pool(name="dqacc", bufs=1) as dqacc,
            tc.tile_pool(name="xp0", bufs=XBUFS[0]) as xp0,
            tc.tile_pool(name="xp1", bufs=XBUFS[1]) as xp1,
            tc.tile_pool(name="xp2", bufs=XBUFS[2]) as xp2,
            tc.tile_pool(name="xp3", bufs=XBUFS[3]) as xp3,
            tc.tile_pool(name="opool", bufs=4) as opool,
            tc.tile_pool(name="ps_l", bufs=2, space="PSUM") as pp_l,
            tc.tile_pool(name="ps_m", bufs=6, space="PSUM") as pp_m,
        ):
            xpools = [xp0, xp1, xp2, xp3]
            u_tiles, s_tiles = {}, {}

            def emit_io(it):
                # u/st ride the Scalar hwdge queue so their triggers never
                # head-of-line-block the x prefetch on the Sync queue
                u = dqio.tile([128, O_SH], F16, tag="u")
                nc.scalar.dma_start(u[:], uc.ap()[:, it, :])
                st = dqio.tile([128, O_SH], F16, tag="st")
                nc.scalar.dma_start(st[:], sc.ap()[:, it, :])
                u_tiles[it], s_tiles[it] = u, st

            emit_io(0)
            emit_io(1)
            la_sb = cpool.tile([R, I], F16, tag="la")
            nc.sync.dma_start(la_sb[:], la.ap())
            lb_sb = cpool.tile([R, O_SH], F16, tag="lb")
            nc.sync.dma_start(lb_sb[:], lb.ap())
            emit_io(2)
            emit_io(3)
            part = cpool.tile([128, MT * O_SH], F16, tag="part")

            wl_tiles = {}

            def emit_fold(it):
                wl = wlpool.tile([128, O_SH], F16, tag="wl")
                pl = pp_l.tile([128, O_SH], F32, tag="pl")
                nc.tensor.matmul(
                    pl[:], la_sb[:, it * 128:(it + 1) * 128], lb_sb[:],
                    start=True, stop=True,
                )
                nc.scalar.copy(wl[:], pl[:])
                wl_tiles[it] = wl

            for it in range(FOLD_UPFRONT):
                emit_fold(it)

            w_aps = {}   # global i-tile -> W tile AP

            def emit_tile(it):
                """Dequant chain (+ fold if not done up-front) for i-tile."""
                if it >= FOLD_UPFRONT:
                    emit_fold(it)
                if it not in u_tiles:
                    emit_io(it)
                u, st = u_tiles.pop(it), s_tiles.pop(it)
                coefs = A5_COEFS if it < N_DEG5_TILES else A_COEFS
                gob = GOB5 if it < N_DEG5_TILES else GOB
                acc = dqacc.tile([128, O_SH], F16, tag="acc")
                acc2 = dqacc.tile([128, O_SH], F16, tag="acc2")
                nc.vector.tensor_scalar_add(acc2[:], u[:], coefs[0])
                nc.vector.tensor_tensor(acc[:], acc2[:], u[:], op=ALU.mult)
                for aj in coefs[1:]:
                    nc.vector.tensor_scalar_add(acc2[:], acc[:], aj)
                    nc.vector.tensor_tensor(acc[:], acc2[:], u[:],
                                            op=ALU.mult)
                # w = (acc + gamma/beta) * (beta*scales) + W_lora
                nc.vector.tensor_scalar_add(acc2[:], acc[:], gob)
                nc.vector.tensor_tensor(acc[:], acc2[:], st[:], op=ALU.mult)
                wt = wpool.tile([128, O_SH], F16, tag="w")
                nc.vector.tensor_tensor(wt[:], acc[:], wl_tiles.pop(it)[:],
                                        op=ALU.add)
                w_aps[it] = wt[:]

            for it in range(4):
                emit_tile(it)
            emit_io(4)
            emit_io(5)

            def issue_xb(ph, mt0):
                i_lo, i_hi = PHASES[ph]
                nb = XBLKS[ph]
                t = xpools[ph].tile([128, nb, i_hi - i_lo, 128], F16,
                                    tag=f"xb{ph}")
                nc.sync.dma_start(
                    t[:], xt.ap()[:, mt0:mt0 + nb, i_lo:i_hi, :]
                )
                return t

            # ---- m-loop in 4 phases over i; partial stays in SBUF f16 ----
            phase_pre = {}   # next-phase x blocks issued ahead of time
            for ph, (i_lo, i_hi) in enumerate(PHASES):
                n_it = i_hi - i_lo
                nb = XBLKS[ph]
                last = ph == len(PHASES) - 1
                pre = phase_pre.pop(ph, [])
                xb = None
                for mt in range(MT):
                    if mt % nb == 0:
                        bi = mt // nb
                        xb = pre[bi] if bi < len(pre) else issue_xb(ph, mt)
                    po = pp_m.tile([128, O_SH], F32, tag="po")
                    for k, it2 in enumerate(range(i_lo, i_hi)):
                        nc.tensor.matmul(
                            po[:], xb[:, mt % nb, k, :], w_aps[it2],
                            start=(k == 0), stop=(k == n_it - 1),
                        )
                    pslice = part[:, mt * O_SH:(mt + 1) * O_SH]
                    if ph == 0:
                        nc.scalar.copy(pslice, po[:])
                    elif not last:
                        nc.vector.tensor_tensor(
                            pslice, po[:], pslice, op=ALU.add
                        )
                    else:
                        ev = opool.tile([128, O_SH], F16, tag="ev")
                        nc.vector.tensor_tensor(
                            ev[:], po[:], pslice, op=ALU.add
                        )
                        nc.scalar.dma_start(
                            out.ap()[mt * 128:(mt + 1) * 128, :], ev[:]
                        )
                    if not last and mt in (40, 44):
                        nxt = ph + 1
                        done = phase_pre.setdefault(nxt, [])
                        done.append(issue_xb(nxt, len(done) * XBLKS[nxt]))
                    for it2, pos in TILE_EMIT.items():
                        if pos == (ph, mt):
                            if it2 + 2 < IT:
                                emit_io(it2 + 2)
                            emit_tile(it2)

    nc.compile()
    return nc


_NC_CACHE = {}


def _get_nc():
    if "nc" not in _NC_CACHE:
        _NC_CACHE["nc"] = _build_nc()
    return _NC_CACHE["nc"]


def prepare_in_maps(x, w_codes, w_scales, lora_a, lora_b):
    """Host-side sharding + layout prep (casts, transposes, constant folds)."""
    xm = np.ascontiguousarray(x.reshape(M, I))
    # xt[p, mt, t, mm] = x[mt*128+mm, t*128+p], f16
    xtl = (
        xm.T.reshape(IT, 128, MT, 128)
        .transpose(1, 2, 0, 3)
        .astype(F16_NP)
    )
    xtl = np.ascontiguousarray(xtl)

    la = np.ascontiguousarray(
        (LORA_SCALE * lora_a.astype(np.float64)).astype(F16_NP)
    )

    # per-i-tile beta (deg-5 head tiles vs deg-9 bulk)
    beta_per_tile = np.full(IT, BETA)
    beta_per_tile[:N_DEG5_TILES] = BETA5

    in_maps = []
    for c in range(N_CORES):
        o_lo, o_hi = c * O_SH, (c + 1) * O_SH
        # u = (2c - 15)/16 in partition-major layout [128, IT, O_SH]
        ut = (2 * w_codes[o_lo:o_hi].astype(np.int32) - 15).T  # [I, O_SH]
        uc_t = np.ascontiguousarray(
            (ut.reshape(IT, 128, O_SH).transpose(1, 0, 2)
             .astype(np.float32) / 16.0).astype(F16_NP)
        )
        # beta * scales, block-expanded, same layout
        s_t = np.repeat(w_scales[o_lo:o_hi].T.astype(np.float64),
                        BLK, axis=0)                           # [I, O_SH]
        s_t = (s_t.reshape(IT, 128, O_SH)
               * beta_per_tile[:, None, None]).transpose(1, 0, 2)
        sc_t = np.ascontiguousarray(s_t.astype(F16_NP))
        lb_t = np.ascontiguousarray(lora_b[o_lo:o_hi].T.astype(F16_NP))
        in_maps.append(
            {
                "xt": xtl,
                "uc": uc_t,
                "sc": sc_t,
                "la": la,
                "lb": lb_t,
            }
        )
    return in_maps


def run(in_maps, trace=False, retries=2):
    nc = _get_nc()
    last = None
    for attempt in range(retries + 1):
        try:
            return run_bass_kernel_spmd(
                nc, in_maps, core_ids=list(range(N_CORES)), trace=trace
            )
        except Exception as e:  # transient NRT/axon device errors
            last = e
            if attempt == retries:
                raise
            import time as _time

            _time.sleep(5)
    raise last


def kernel(x, w_codes, w_scales, lora_a, lora_b):
    in_maps = prepare_in_maps(x, w_codes, w_scales, lora_a, lora_b)
    res = run(in_maps, trace=False)
    out = np.concatenate(
        [res.results[c]["out"] for c in range(N_CORES)], axis=1
    )
    return out.reshape(B, S, O).astype(np.float32)
